# revision 7
# baseline (speedup 1.0000x reference)
"""v4: dma_gather (Q7 SWDGE gather) with 4-way class split.

indirect_dma_start costs ~1us of Pool-engine descriptor generation per
instruction and only carries 128 descriptors ([128,1] offsets), which made
the baseline ~768us. dma_gather carries up to 2048 rows per instruction.

Its int16 indices only reach 32767, so rows are split by class b=(r+4)&3
with per-class base offset: the table is uploaded as T[32769, 512] fp32
(stride 2048B = 4 rows per super-row, 4 zero rows prepended), and class b
gathers from column block b with index q=(r+4)>>2 <= 32767. Pad slots point
at q=0 (a zero row), so one tensor_reduce per tile sums everything.
Rows r>=131068 (unreachable by int16 q) are remapped to the zero row and
added back via a host-computed per-node correction uploaded with the scales.
"""
import os
import sys

for _p in ("/opt/trn_rl_repo", "/opt/pypackages"):
    if _p not in sys.path and os.path.isdir(_p):
        sys.path.append(_p)

import numpy as np

NUM_AUTHOR = 131072
D = 128
N_NODES = 32768
G = 32
NCORES = 8
NPC = N_NODES // NCORES   # 4096
P = 128
TILES = NPC // P          # 32
NCLS = 4
SHIFT = 4                 # zero rows prepended; u = r + SHIFT
QMAX = 32767
TROWS = (SHIFT + NUM_AUTHOR) // NCLS   # 32769 super-rows of 4 rows
LCHUNK = 8                # max gather columns per instruction (ring limit)

_CACHE = {}
LAST_RESULT = None


def _plan(lengths, neighbors):
    """Host-side plan: per-core sort order, per-(tile,class) column counts,
    and per-core class-split neighbor lists (as q indices)."""
    lengths = np.asarray(lengths).reshape(NCORES, NPC)
    neighbors = np.asarray(neighbors).reshape(NCORES, NPC, G)
    orders = []
    qlists = []   # qlists[c][t][b] = [P] list of arrays of q values
    counts = np.zeros((NCORES, TILES, NCLS), dtype=np.int64)
    corr_refs = []  # per core: list of (sorted_node_idx, row r) for r>=limit
    rmax = NCLS * QMAX + SHIFT - 1 - SHIFT  # max reachable original row
    for c in range(NCORES):
        order = np.argsort(-lengths[c], kind="stable")
        orders.append(order)
        nb = neighbors[c][order]
        ln = lengths[c][order]
        qs = [[None] * NCLS for _ in range(TILES)]
        refs = []
        for t in range(TILES):
            for b in range(NCLS):
                qs[t][b] = [None] * P
        for t in range(TILES):
            sl_nb = nb[t * P:(t + 1) * P]
            sl_ln = ln[t * P:(t + 1) * P]
            for p in range(P):
                row = sl_nb[p, :sl_ln[p]].astype(np.int64)
                big = row > rmax
                for r in row[big]:
                    refs.append((t * P + p, int(r)))
                row = row[~big]
                u = row + SHIFT
                b_ = u & (NCLS - 1)
                q_ = u >> 2
                for b in range(NCLS):
                    qb = q_[b_ == b]
                    qs[t][b][p] = qb.astype(np.int16)
                    counts[c, t, b] = max(counts[c, t, b], len(qb))
        qlists.append(qs)
        corr_refs.append(refs)
    tile_cls = counts.max(axis=0)  # [TILES, NCLS] cross-core col counts
    return orders, qlists, tile_cls, corr_refs


def _instr_layout(tile_cls):
    """Per-tile column layout and per-instruction chunks.

    Returns: cols[t] total columns of tile t; instr = list of
    (t, b, col0_in_tile, L) in issue order; csum_cols[t] column base."""
    instr = []
    cols = []
    for t in range(TILES):
        off = 0
        for b in range(NCLS):
            L = int(tile_cls[t, b])
            while L > 0:
                Lc = min(L, LCHUNK)
                instr.append((t, b, off, Lc))
                off += Lc
                L -= Lc
        cols.append(off)
    return cols, instr


def _build_program(cols, instr):
    from concourse import bacc, bass, mybir
    from concourse import library_config

    nc = bacc.Bacc("TRN2", target_bir_lowering=False, debug=False,
                   enable_asserts=False, num_devices=NCORES,
                   dynamic_dma_scratch_size=2**15)
    dt = mybir.dt
    maxcols = max(max(cols), 1)
    # int16 index columns per instruction: 8 * L
    idx16_total = sum(8 * L for (_, _, _, L) in instr)
    T = nc.dram_tensor("tab", [TROWS, NCLS * D], dt.float32, kind="ExternalInput")
    idx = nc.dram_tensor("idx", [P, idx16_total], dt.int16, kind="ExternalInput")
    scl = nc.dram_tensor("scl", [P, TILES], dt.float32, kind="ExternalInput")
    corr = nc.dram_tensor("corr", [P, TILES * D], dt.float32, kind="ExternalInput")
    out = nc.dram_tensor("out", [NPC, D], dt.float32, kind="ExternalOutput")

    # per-instruction idx offset and per-(parity) cumulative instr counts
    ioff = []
    o = 0
    for (_, _, _, L) in instr:
        ioff.append(o)
        o += 8 * L
    # instructions grouped by tile
    by_tile = [[] for _ in range(TILES)]
    for i, (t, b, c0, L) in enumerate(instr):
        by_tile[t].append(i)
    cum_par = {0: [0] * TILES, 1: [0] * TILES}
    tot = {0: 0, 1: 0}
    for t in range(TILES):
        tot[t % 2] += len(by_tile[t])
        cum_par[t % 2][t] = tot[t % 2]

    with (
        nc.Block() as block,
        nc.sbuf_tensor("idx_sb", [P, idx16_total], dt.int16) as idx_sb,
        nc.sbuf_tensor("scl_sb", [P, TILES], dt.float32) as scl_sb,
        nc.sbuf_tensor("corr_sb", [P, TILES * D], dt.float32) as corr_sb,
        nc.sbuf_tensor("g0", [P, maxcols * D], dt.float32) as g0,
        nc.sbuf_tensor("g1", [P, maxcols * D], dt.float32) as g1,
        nc.sbuf_tensor("r0", [P, D], dt.float32) as r0,
        nc.sbuf_tensor("r1", [P, D], dt.float32) as r1,
        nc.semaphore("iosem") as iosem,
        nc.semaphore("dsem0") as dsem0,
        nc.semaphore("dsem1") as dsem1,
        nc.semaphore("rsem") as rsem,
        nc.semaphore("wsem0") as wsem0,
        nc.semaphore("wsem1") as wsem1,
    ):
        gbuf = [g0, g1]
        rbuf = [r0, r1]
        dsem = [dsem0, dsem1]
        wsem = [wsem0, wsem1]

        @block.sync
        def _(sync):
            sync.dma_start(out=idx_sb[:], in_=idx[:]).then_inc(iosem, 16)
            sync.dma_start(out=scl_sb[:], in_=scl[:]).then_inc(iosem, 16)
            sync.dma_start(out=corr_sb[:], in_=corr[:]).then_inc(iosem, 16)
            for t in range(TILES):
                sync.wait_ge(rsem, t + 1)
                sync.dma_start(
                    out=out[t * P:(t + 1) * P, :], in_=rbuf[t % 2][:]
                ).then_inc(wsem[t % 2], 16)
            sync.wait_ge(wsem0, 16 * (TILES // 2))
            sync.wait_ge(wsem1, 16 * (TILES // 2))

        @block.gpsimd
        def _(gpsimd):
            gpsimd.load_library(library_config.mlp)
            gpsimd.wait_ge(iosem, 48)
            for t in range(TILES):
                if t >= 2:
                    gpsimd.wait_ge(rsem, t - 1)  # g[t%2] free after reduce t-2
                par = t % 2
                for i in by_tile[t]:
                    _, b, c0, L = instr[i]
                    n_idx = P * L
                    dest = (gbuf[par][:, c0 * D:(c0 + L) * D]
                            .rearrange("p (g d) -> p g d", g=L, d=D))
                    gpsimd.dma_gather(
                        dest,
                        T[:, b * D:(b + 1) * D],
                        idx_sb[:, ioff[i]:ioff[i] + 8 * L],
                        n_idx,
                        n_idx,
                        D,
                        elem_step=NCLS * D,
                    ).then_inc(dsem[par], 16)

        @block.vector
        def _(vector):
            vector.wait_ge(iosem, 48)
            for t in range(TILES):
                par = t % 2
                vector.wait_ge(dsem[par], 16 * cum_par[par][t])
                if t >= 2:
                    vector.wait_ge(wsem[par], 16 * (t // 2))  # r[par] free
                ct = max(cols[t], 1)
                gv = (gbuf[par][:, 0:ct * D]
                      .rearrange("p (g d) -> p d g", g=ct, d=D))
                vector.tensor_reduce(
                    out=rbuf[par][:], in_=gv,
                    axis=mybir.AxisListType.X, op=mybir.AluOpType.add,
                )
                sv = scl_sb[:, t:t + 1].broadcast_to([P, D])
                vector.tensor_tensor(
                    out=rbuf[par][:], in0=rbuf[par][:], in1=sv,
                    op=mybir.AluOpType.mult,
                )
                vector.tensor_tensor(
                    out=rbuf[par][:], in0=rbuf[par][:],
                    in1=corr_sb[:, t * D:(t + 1) * D],
                    op=mybir.AluOpType.add,
                ).then_inc(rsem, 1)

    nc.compile()
    return nc


def _prep_inputs(lengths, a2e, orders, qlists, tile_cls, corr_refs, instr):
    lengths = np.asarray(lengths).reshape(NCORES, NPC)
    a2e = np.asarray(a2e, dtype=np.float32)

    # table: 4 zero rows + a2e, viewed as [TROWS, 4*D]
    tab = np.zeros((SHIFT + NUM_AUTHOR, D), dtype=np.float32)
    tab[SHIFT:] = a2e
    tab = np.ascontiguousarray(tab.reshape(TROWS, NCLS * D))

    idx16_total = sum(8 * L for (_, _, _, L) in instr)
    idx_dram = np.zeros((NCORES, 16, idx16_total), dtype=np.int16)
    scl_dram = np.zeros((NCORES, P, TILES), dtype=np.float32)
    corr_dram = np.zeros((NCORES, P, TILES * D), dtype=np.float32)

    for c in range(NCORES):
        ln = lengths[c][orders[c]]
        inv = np.where(ln > 0, 1.0 / np.maximum(ln, 1), 0.0).astype(np.float32)
        for t in range(TILES):
            scl_dram[c, :, t] = inv[t * P:(t + 1) * P]
        for sp, r in corr_refs[c]:
            t, p = divmod(sp, P)
            corr_dram[c, p, t * D:(t + 1) * D] += a2e[r] * inv[sp]

        off = 0
        consumed = np.zeros((TILES, NCLS), dtype=np.int64)
        for (t, b, c0, L) in instr:
            # flat k = col*128 + p over this chunk's L columns
            block16 = np.zeros((16, 8 * L), dtype=np.int16)
            qs = qlists[c][t][b]
            start = consumed[t, b]
            flat = np.zeros(P * L, dtype=np.int16)
            for p in range(P):
                qv = qs[p]
                seg = qv[start:start + L]
                if len(seg):
                    kcols = np.arange(len(seg))
                    flat[kcols * P + p] = seg
            block16 = flat.reshape(8 * L, 16).T.astype(np.int16)
            idx_dram[c, :, off:off + 8 * L] = block16
            consumed[t, b] += L
            off += 8 * L

    idx_dram_full = np.tile(idx_dram, (1, 8, 1))  # replicate to 128 partitions
    return tab, idx_dram_full, scl_dram, corr_dram


def _install_ntff_hook_shim():
    import types
    if "antenv.axon_hooks" in sys.modules:
        return
    from trn_agent_boot.trn_boot import _ntff_profile_via_ctypes
    hook = _ntff_profile_via_ctypes("/opt/axon/libaxon_pjrt.so")
    mod = types.ModuleType("antenv.axon_hooks")
    mod._hook = hook
    mod.get_axon_ntff_profile_hook = lambda: mod._hook
    mod.set_axon_ntff_profile_hook = lambda h: setattr(mod, "_hook", h)
    sys.modules["antenv.axon_hooks"] = mod


def kernel(node, neighbors, lengths, a2e, _trace=False):
    global LAST_RESULT
    from concourse.bass_utils import run_bass_kernel_spmd

    if _trace:
        try:
            _install_ntff_hook_shim()
            import concourse.bass_utils as _bu
            _bu.upload_artifacts = lambda tmpdir: f"local://{tmpdir}"
        except Exception as e:
            print(f"ntff hook shim failed ({e}); running without trace")
            _trace = False

    orders, qlists, tile_cls, corr_refs = _plan(lengths, neighbors)
    cols, instr = _instr_layout(tile_cls)
    key = tuple(tuple(x) for x in tile_cls)
    if _CACHE.get("key") != key:
        _CACHE["nc"] = _build_program(cols, instr)
        _CACHE["key"] = key
    nc = _CACHE["nc"]

    tab, idx_dram, scl_dram, corr_dram = _prep_inputs(
        lengths, a2e, orders, qlists, tile_cls, corr_refs, instr)
    in_maps = [
        {
            "tab": tab,
            "idx": np.ascontiguousarray(idx_dram[c]),
            "scl": np.ascontiguousarray(scl_dram[c]),
            "corr": np.ascontiguousarray(corr_dram[c]),
        }
        for c in range(NCORES)
    ]
    res = run_bass_kernel_spmd(nc, in_maps, list(range(NCORES)), trace=_trace)
    LAST_RESULT = res

    final = np.empty((N_NODES, D), dtype=np.float32)
    for c in range(NCORES):
        block = final[c * NPC:(c + 1) * NPC]
        block[orders[c]] = res.results[c]["out"]
    return final


# revision 9
# speedup vs baseline: 1.8558x; 1.8558x over previous
"""v5: baseline indirect gathers round-robined across 4 SWDGE queues.

Nodes are sorted by degree (desc) per core so each 128-node tile only
gathers max-degree-in-tile neighbor columns (~half the slots are padding
in the unsorted layout). Raw Bass Block avoids per-call Tile sync cost.
"""
import os
import sys

for _p in ("/opt/trn_rl_repo", "/opt/pypackages"):
    if _p not in sys.path and os.path.isdir(_p):
        sys.path.append(_p)

import numpy as np

NUM_AUTHOR = 131072
D = 128
N_NODES = 32768
G = 32
NCORES = 8
NPC = N_NODES // NCORES   # 4096
P = 128
TILES = NPC // P          # 32
ZERO_ROW = NUM_AUTHOR

_CACHE = {}
LAST_RESULT = None


def _tile_maxlens(lengths):
    """Per-core sort order and per-tile gather column counts (compile-time)."""
    lengths = np.asarray(lengths).reshape(NCORES, NPC)
    orders, tlens = [], []
    for c in range(NCORES):
        order = np.argsort(-lengths[c], kind="stable")
        lens_sorted = lengths[c][order]
        lt = [max(int(lens_sorted[t * P]), 1) for t in range(TILES)]
        orders.append(order)
        tlens.append(lt)
    return orders, tlens


def _build_program(tile_lens):
    """tile_lens: [TILES] ints — max over cores of each tile's column count
    (SPMD: one program for all cores)."""
    from concourse import bacc, bass, mybir

    nc = bacc.Bacc("TRN2", target_bir_lowering=False, debug=False,
                   enable_asserts=False, num_devices=NCORES,
                   num_swdge_queues=4)
    dt = mybir.dt
    ctotal = sum(tile_lens)
    a2e = nc.dram_tensor("a2e", [NUM_AUTHOR + 1, D], dt.float32, kind="ExternalInput")
    idx = nc.dram_tensor("idx", [P, ctotal], dt.int32, kind="ExternalInput")
    scl = nc.dram_tensor("scl", [P, TILES], dt.float32, kind="ExternalInput")
    out = nc.dram_tensor("out", [NPC, D], dt.float32, kind="ExternalOutput")

    csum = [0]
    for L in tile_lens:
        csum.append(csum[-1] + L)

    with (
        nc.Block() as block,
        nc.sbuf_tensor("idx_sb", [P, ctotal], dt.int32) as idx_sb,
        nc.sbuf_tensor("scl_sb", [P, TILES], dt.float32) as scl_sb,
        nc.sbuf_tensor("g0", [P, G * D], dt.float32) as g0,
        nc.sbuf_tensor("g1", [P, G * D], dt.float32) as g1,
        nc.sbuf_tensor("r0", [P, D], dt.float32) as r0,
        nc.sbuf_tensor("r1", [P, D], dt.float32) as r1,
        nc.semaphore("iosem") as iosem,
        nc.semaphore("dsem0") as dsem0,
        nc.semaphore("dsem1") as dsem1,
        nc.semaphore("rsem") as rsem,
        nc.semaphore("esem") as esem,
        nc.semaphore("wsem0") as wsem0,
        nc.semaphore("wsem1") as wsem1,
    ):
        gbuf = [g0, g1]
        rbuf = [r0, r1]
        dsem = [dsem0, dsem1]
        wsem = [wsem0, wsem1]
        # cumulative gather-call counts per tile parity
        cumpar = {0: [], 1: []}
        tot = {0: 0, 1: 0}
        for t, L in enumerate(tile_lens):
            tot[t % 2] += L
            cumpar[t % 2].append(tot[t % 2])

        @block.sync
        def _(sync):
            sync.dma_start(out=idx_sb[:], in_=idx[:]).then_inc(iosem, 16)
            sync.dma_start(out=scl_sb[:], in_=scl[:]).then_inc(iosem, 16)
            for t in range(TILES):
                sync.wait_ge(rsem, t + 1)
                sync.dma_start(
                    out=out[t * P:(t + 1) * P, :], in_=rbuf[t % 2][:]
                ).then_inc(wsem[t % 2], 16)
            sync.wait_ge(wsem0, 16 * (TILES // 2))
            sync.wait_ge(wsem1, 16 * (TILES // 2))

        @block.gpsimd
        def _(gpsimd):
            gpsimd.wait_ge(iosem, 32)  # idx + scl loaded
            for t in range(TILES):
                if t >= 2:
                    gpsimd.wait_ge(rsem, t - 1)  # g[t%2] free after reduce t-2
                for j in range(tile_lens[t]):
                    c = csum[t] + j
                    inst = gpsimd.indirect_dma_start(
                        out=gbuf[t % 2][:, j * D:(j + 1) * D],
                        out_offset=None,
                        in_=a2e[:],
                        in_offset=bass.IndirectOffsetOnAxis(
                            ap=idx_sb[:, c:c + 1], axis=0,
                        ),
                    )
                    inst.then_inc(dsem[t % 2], 16)
                    qi = c % 4
                    inst.ins.queue = f"qPoolDynamic{qi or ''}"

        @block.vector
        def _(vector):
            vector.wait_ge(iosem, 32)  # scl loaded
            for t in range(TILES):
                vector.wait_ge(dsem[t % 2], 16 * cumpar[t % 2][t // 2])
                if t >= 2:
                    vector.wait_ge(wsem[t % 2], 16 * (t // 2))  # r[t%2] free
                L = tile_lens[t]
                gv = (gbuf[t % 2][:]
                      .rearrange("p (g d) -> p d g", g=G, d=D)[:, :, 0:L])
                vector.tensor_reduce(
                    out=rbuf[t % 2][:], in_=gv,
                    axis=mybir.AxisListType.X, op=mybir.AluOpType.add,
                ).then_inc(esem, 1)
                vector.wait_ge(esem, t + 1)
                sv = scl_sb[:, t:t + 1].broadcast_to([P, D])
                vector.tensor_tensor(
                    out=rbuf[t % 2][:], in0=rbuf[t % 2][:], in1=sv,
                    op=mybir.AluOpType.mult,
                ).then_inc(rsem, 1)

    nc.compile()
    return nc


def _prep_inputs(neighbors, lengths, a2e, orders, tile_lens):
    neighbors = np.asarray(neighbors).reshape(NCORES, NPC, G)
    lengths = np.asarray(lengths).reshape(NCORES, NPC)
    a2e = np.asarray(a2e, dtype=np.float32)
    ctotal = sum(tile_lens)

    idx_dram = np.full((NCORES, P, ctotal), ZERO_ROW, dtype=np.int32)
    scl_dram = np.zeros((NCORES, P, TILES), dtype=np.float32)
    for c in range(NCORES):
        order = orders[c]
        nb = neighbors[c][order]          # [NPC, G] sorted
        ln = lengths[c][order]            # [NPC]
        mask = np.arange(G)[None, :] < ln[:, None]
        nbc = np.where(mask, nb, ZERO_ROW).astype(np.int32)
        inv = np.where(ln > 0, 1.0 / np.maximum(ln, 1), 0.0).astype(np.float32)
        off = 0
        for t in range(TILES):
            L = tile_lens[t]
            idx_dram[c, :, off:off + L] = nbc[t * P:(t + 1) * P, :L]
            scl_dram[c, :, t] = inv[t * P:(t + 1) * P]
            off += L
    a2e_pad = np.concatenate([a2e, np.zeros((1, D), np.float32)], axis=0)
    return idx_dram, scl_dram, a2e_pad


def _install_ntff_hook_shim():
    import types
    if "antenv.axon_hooks" in sys.modules:
        return
    from trn_agent_boot.trn_boot import _ntff_profile_via_ctypes
    hook = _ntff_profile_via_ctypes("/opt/axon/libaxon_pjrt.so")
    mod = types.ModuleType("antenv.axon_hooks")
    mod._hook = hook
    mod.get_axon_ntff_profile_hook = lambda: mod._hook
    mod.set_axon_ntff_profile_hook = lambda h: setattr(mod, "_hook", h)
    sys.modules["antenv.axon_hooks"] = mod


def kernel(node, neighbors, lengths, a2e, _trace=False):
    global LAST_RESULT
    from concourse.bass_utils import run_bass_kernel_spmd

    if _trace:
        try:
            _install_ntff_hook_shim()
            import concourse.bass_utils as _bu
            _bu.upload_artifacts = lambda tmpdir: f"local://{tmpdir}"
        except Exception as e:
            print(f"ntff hook shim failed ({e}); running without trace")
            _trace = False

    orders, percore_lens = _tile_maxlens(lengths)
    tile_lens = [max(percore_lens[c][t] for c in range(NCORES))
                 for t in range(TILES)]
    key = tuple(tile_lens)
    if _CACHE.get("key") != key:
        _CACHE["nc"] = _build_program(tile_lens)
        _CACHE["key"] = key
    nc = _CACHE["nc"]

    idx_dram, scl_dram, a2e_pad = _prep_inputs(
        neighbors, lengths, a2e, orders, tile_lens)
    in_maps = [
        {
            "a2e": np.ascontiguousarray(a2e_pad),
            "idx": np.ascontiguousarray(idx_dram[c]),
            "scl": np.ascontiguousarray(scl_dram[c]),
        }
        for c in range(NCORES)
    ]
    res = run_bass_kernel_spmd(nc, in_maps, list(range(NCORES)), trace=_trace)
    LAST_RESULT = res

    final = np.empty((N_NODES, D), dtype=np.float32)
    for c in range(NCORES):
        block = final[c * NPC:(c + 1) * NPC]
        block[orders[c]] = res.results[c]["out"]
    return final

